# revision 5
# baseline (speedup 1.0000x reference)
"""GPT causal attention block (B=2, S=2048, H=16, hd=64, d=1024),
sharded over 8 NeuronCores as (batch x head-group): core c -> batch c//4,
heads 4*(c%4) .. 4*(c%4)+3.  All matmuls in bf16 (fp32 PSUM accumulate).

Per-core device program:
  ph1: qkT = (Wqk*[1/8,1]).T @ x^T  [512, 2048] bf16 (q cols pre-scaled 1/8)
       v   = x @ Wv + bv            stored ones-augmented [128,16,4,65] bf16
  ph2 (O-layout): per (qchunk c4, head h):
       per k-block pair (2j, 2j+1), sharing one 2-bank PSUM tile:
         ST  = kT_h[j].T @ qT_h     [128 ktok, <=512 qtok] PSUM
         ST += I.T @ maskw          (-60 on masked diag region, via matmul)
         PT  = exp(ST)              bf16 SBUF (ACT engine, one exp per pair)
       per q-block qb (one strictly-sequential PSUM group per bank):
         O[qb] = sum_j PT_j[:, qb].T @ v_aug_j   [128 qtok, 65], col 64 = l
         att = O * (1/l)            DVE recip + per-partition tensor_scalar
  ph1 chunks 1..3 and ph3 chunks 0..2 are emitted interleaved into ph2 as PE
  filler (late ph2 windows are exp/ACT-bound; PE runs projections there).
  xbar: attT = att.T via XBAR DMA-transpose (no compute-engine cost)
  ph3: out = attT.T @ Wo -> bf16 out (bias bo and partial-sum on host)
Host sums the 4 row-parallel partials per batch and adds bo.
"""
import sys
import numpy as np

sys.path.insert(0, "/opt/trn_rl_repo")

import concourse.bass as bass
import concourse.mybir as mybir
import concourse.tile as tile

B, S, D, NH, HD = 2, 2048, 1024, 16, 64
HPC = 4            # heads per core
NKB = S // 128     # 16 k-blocks
NQC = S // 512     # 4 q-chunks
F32 = mybir.dt.float32
BF16 = mybir.dt.bfloat16
MASK_NEG = -60.0
MAX_WAITS = 1      # one sync-wait per NoOp; walrus limits are per-engine and tight


def _split_excess_waits(nc, max_waits=MAX_WAITS):
    """walrus CoreV3 rejects instructions with more than ~4 sync waits; move
    the excess onto same-engine NoOps inserted just before the instruction."""
    n_split = 0
    for blk in nc.m.functions[0].blocks:
        for idx in range(len(blk.instructions) - 1, -1, -1):
            inst = blk.instructions[idx]
            if isinstance(inst, mybir.InstISA) and inst.isa_opcode == 176:
                # EVENT_SEMAPHORE_RANGE_CLEAR mis-encodes for this walrus
                # ("ISA wrong length"); sems are re-zeroed by NRT per load.
                blk.instructions.pop(idx)
        idx = 0
        while idx < len(blk.instructions):
            inst = blk.instructions[idx]
            si = inst.sync_info
            lim = 0 if isinstance(inst, mybir.InstMatmult) else max_waits
            if si is not None and si.on_wait and len(si.on_wait) > lim:
                waits = list(si.on_wait)
                si.on_wait = waits[len(waits) - lim:] if lim else []
                rest = waits[:len(waits) - lim] if lim else waits
                for i in range(0, len(rest), max_waits):
                    nop = mybir.InstNoOp(
                        name=nc.get_next_instruction_name(),
                        sync_info=mybir.SyncInfo(
                            on_wait=rest[i:i + max_waits], on_update=[]
                        ),
                        bass_nofuse=True,
                        engine=inst.engine,
                    )
                    nc.register_instruction(nop)
                    blk.instructions.insert(idx, nop)
                    idx += 1
                n_split += 1
            idx += 1
    return n_split


def _build():
    nc = bass.Bass("TRN2", target_bir_lowering=False, debug=False, num_devices=8)
    xT = nc.declare_dram_parameter("xT", [D, S], BF16, isOutput=False)
    wqk = nc.declare_dram_parameter("wqk", [D, 512], BF16, isOutput=False)
    wv = nc.declare_dram_parameter("wv", [D, 256], BF16, isOutput=False)
    bqk = nc.declare_dram_parameter("bqk", [512], F32, isOutput=False)
    bv = nc.declare_dram_parameter("bv", [256], F32, isOutput=False)
    wo = nc.declare_dram_parameter("wo", [256, D], BF16, isOutput=False)
    ident = nc.declare_dram_parameter("ident", [128, 128], BF16, isOutput=False)
    maskw = nc.declare_dram_parameter("maskw", [128, 512], BF16, isOutput=False)
    out = nc.declare_dram_parameter("out", [S, D], BF16, isOutput=True)

    with tile.TileContext(nc) as tc:
        with (
            tc.tile_pool(name="singles", bufs=1) as singles,
            tc.tile_pool(name="xtp", bufs=2) as xtp,
            tc.tile_pool(name="pt", bufs=18) as ptp,
            tc.tile_pool(name="rl", bufs=4) as rlp,
        ):
            # ---- resident SBUF tensors ----
            wqk_sb = singles.tile([128, 8, 512], BF16)
            wv_sb = singles.tile([128, 8, 256], BF16)
            wo_sb = singles.tile([128, 2, D], BF16)
            qT_sb = singles.tile([128, 2, S], BF16)        # q, heads pair-stacked
            kT_sb = singles.tile([128, 2, S], BF16)
            v_sb = singles.tile([128, NKB, HPC, 65], BF16)  # ones-augmented v
            att_sb = singles.tile([128, NKB, 256], BF16)   # [tok, tb, h*64+e]
            attT_sb = singles.tile([128, 2, S], BF16)      # [feat, fb, tok]
            bqk_sb = singles.tile([128, 4], F32)           # per-feat-block bias col
            bv_sb = singles.tile([128, 256], F32)          # bv partition-bcast
            ident_sb = singles.tile([128, 128], BF16)
            maskw_sb = singles.tile([128, 512], BF16)

            # ---- phase-1 units (emitted directly for chunk 0, as ph2 filler
            # for chunks 1..3) ----
            ph1pool = tc.tile_pool(name="ps_qkv", bufs=2, space="PSUM")
            ps_qkv = ph1pool.__enter__()
            xts = {}

            def emit_x_dma(t):
                xt = xtp.tile([128, 8, 512], BF16, tag=f"xt{t % 2}")
                nc.sync.dma_start(
                    out=xt,
                    in_=xT[:, t * 512:(t + 1) * 512].rearrange(
                        "(db p) s -> p db s", p=128),
                )
                xts[t] = xt

            # critical-path first: qk weights for the first three feature
            # blocks and x chunk 0 as single merged DMAs (HWDGE per-DMA
            # overhead dominates the cold start), then the rest.
            nc.sync.dma_start(
                out=wqk_sb[:, :, 0:384],
                in_=wqk[:, 0:384].rearrange("(db p) c -> p db c", p=128))
            xt0 = xtp.tile([128, 8, 512], BF16, tag="xt0")
            nc.sync.dma_start(
                out=xt0,
                in_=xT[:, 0:512].rearrange("(db p) s -> p db s", p=128))
            xts[0] = xt0
            nc.sync.dma_start(
                out=wqk_sb[:, :, 384:512],
                in_=wqk[:, 384:512].rearrange("(db p) c -> p db c", p=128))
            nc.sync.dma_start(out=bqk_sb, in_=bqk[:].rearrange("(blk p) -> p blk", p=128))
            nc.sync.dma_start(
                out=wv_sb, in_=wv[:, :].rearrange("(db p) c -> p db c", p=128))
            nc.sync.dma_start(
                out=bv_sb,
                in_=bass.AP(tensor=bv[:].tensor, offset=bv[:].offset, ap=[[0, 128], [1, 256]]),
            )
            nc.sync.dma_start(out=ident_sb, in_=ident[:, :])
            nc.sync.dma_start(out=maskw_sb, in_=maskw[:, :])
            nc.sync.dma_start(
                out=wo_sb, in_=wo[:, :].rearrange("(fb p) c -> p fb c", p=128))
            nc.vector.memset(v_sb[:, :, :, 64:65], 1.0)

            def emit_qk_unit(t, fb, half=None):
                # fb 0,1: q pair-blocks; 2,3: k pair-blocks
                xt = xts[t]
                sl = slice(0, 512) if half is None else slice(half * 256, half * 256 + 256)
                ps = ps_qkv.tile([128, 512], F32, tag="u")
                for d in range(8):
                    nc.tensor.matmul(
                        ps[:, sl], wqk_sb[:, d, fb * 128:(fb + 1) * 128], xt[:, d, sl],
                        start=(d == 0), stop=(d == 7),
                    )
                dst = (qT_sb if fb < 2 else kT_sb)[
                    :, fb % 2, t * 512 + sl.start:t * 512 + sl.stop]
                nc.vector.tensor_scalar_add(dst, ps[:, sl], bqk_sb[:, fb:fb + 1])

            def emit_v_unit(t, tb):
                xt = xts[t]
                psu = ps_qkv.tile([128, 512], F32, tag="u")
                psv = psu[:, 0:256]
                for d in range(8):
                    nc.tensor.matmul(
                        psv, xt[:, d, tb * 128:(tb + 1) * 128], wv_sb[:, d, :],
                        start=(d == 0), stop=(d == 7),
                    )
                kb = t * 4 + tb
                nc.vector.tensor_add(
                    out=v_sb[:, kb, :, 0:64],
                    in0=psv.rearrange("p (h e) -> p h e", h=4),
                    in1=bv_sb.rearrange("p (h e) -> p h e", h=4),
                )

            # ---- phase-2 pools ----
            ph2a = tc.tile_pool(name="ps_st", bufs=2, space="PSUM")
            ph2b = tc.tile_pool(name="ps_o", bufs=2, space="PSUM")
            ps_st = ph2a.__enter__()
            ps_o = ph2b.__enter__()

            def emit_pair(c4, h, ja, jb):
                """ST+exp for k-blocks (ja, jb) against q-chunk c4 of head h.
                Returns the pt tile holding exp(scores)."""
                hp, hb = h // 2, (h % 2) * 64
                q_lo = c4 * 512
                los = []
                st = ps_st.tile([128, 2, 512], F32)
                for jj, j in ((0, ja), (1, jb)):
                    m = j - 4 * c4
                    lo = max(m, 0) * 128
                    los.append(lo)
                    nc.tensor.matmul(
                        st[:, jj, lo:512],
                        kT_sb[hb:hb + 64, hp, j * 128:(j + 1) * 128],
                        qT_sb[hb:hb + 64, hp, q_lo + lo:q_lo + 512],
                        start=True, stop=(m < 0),
                    )
                    if m >= 0:
                        # accumulate -60 onto the masked (strictly-lower) part
                        # of the diagonal 128x128 sub-block; exp then yields
                        # ~e^-60, i.e. an effective causal mask.
                        nc.tensor.matmul(
                            st[:, jj, lo:lo + 128], ident_sb, maskw_sb[:, 0:128],
                            start=False, stop=True,
                        )
                pt = ptp.tile([128, 2, 512], BF16)
                lo = min(los)
                nc.scalar.activation(
                    out=pt[:, :, lo:512], in_=st[:, :, lo:512],
                    func=mybir.ActivationFunctionType.Exp,
                )
                return pt

            def emit_o_qb(c4, h, qb, pts):
                """One PSUM accumulation group: O[qb] over all j <= 4*c4+qb.
                Strictly sequential group in its own bank (HW requirement)."""
                last_j = 4 * c4 + qb
                po = ps_o.tile([128, 65], F32)
                for j in range(last_j + 1):
                    pt = pts[j // 2]
                    nc.tensor.matmul(
                        po,
                        pt[:, j % 2, qb * 128:(qb + 1) * 128],
                        v_sb[:, j, h, :],
                        start=(j == 0), stop=(j == last_j),
                    )
                rl = rlp.tile([128, 1], F32)
                nc.vector.reciprocal(rl, po[:, 64:65])
                nc.vector.tensor_scalar_mul(
                    att_sb[:, 4 * c4 + qb, h * 64:(h + 1) * 64],
                    po[:, 0:64],
                    rl[:, 0:1],
                )

            def emit_xbar(tb, fb):
                nc.sync.dma_start_transpose(
                    out=attT_sb[:, fb, tb * 128:(tb + 1) * 128],
                    in_=att_sb[:, tb, fb * 128:(fb + 1) * 128],
                )

            zsp = tc.tile_pool(name="zs", bufs=4)
            zs_pool = zsp.__enter__()

            def emit_ph3_unit(tb, last):
                zs = zs_pool.tile([128, 2, 512], BF16)
                for oc in range(2):
                    ps = ps_qkv.tile([128, 512], F32, tag="u")
                    for fb in range(2):
                        nc.tensor.matmul(
                            ps, attT_sb[:, fb, tb * 128:(tb + 1) * 128],
                            wo_sb[:, fb, oc * 512:(oc + 1) * 512],
                            start=(fb == 0), stop=(fb == 1),
                        )
                    if last and oc == 1:
                        nc.scalar.copy(zs[:, oc, :], ps)
                    else:
                        nc.vector.tensor_copy(zs[:, oc, :], ps)
                nc.sync.dma_start(
                    out=out[tb * 128:(tb + 1) * 128, :],
                    in_=zs.rearrange("p a b -> p (a b)"),
                )

            # ---- schedule ----
            # chunk 0 qk at half-token granularity: the first attention pairs
            # only need the first half projected, so they start ~4us earlier.
            for fb, half in ((0, 0), (2, 0), (0, 1), (2, 1), (1, 0), (3, 0), (1, 1), (3, 1)):
                emit_qk_unit(0, fb, half)
            for tb in range(4):
                emit_v_unit(0, tb)
            emit_x_dma(1)

            # PE filler units dripped into each ACT-bound ph2 chunk:
            #   chunk 0,1: the qkv units of the next chunk
            #   chunk 2:   qkv units of chunk 3 + out-proj of chunks 0,1 (part)
            #   chunk 3:   out-proj through chunk 2
            # out-proj of chunk 3 runs at the end.
            for c4 in range(NQC):
                fillers = []
                if c4 + 2 < NQC:
                    emit_x_dma(c4 + 2)
                if c4 + 1 < NQC:
                    fillers += [(emit_qk_unit, (c4 + 1, fb)) for fb in range(4)]
                    fillers += [(emit_v_unit, (c4 + 1, tb)) for tb in range(4)]
                if c4 == 2:
                    fillers += [(emit_ph3_unit, (tb, False)) for tb in range(4)]
                elif c4 == 3:
                    fillers += [(emit_ph3_unit, (tb, False)) for tb in range(4, 12)]
                n_pairs = (2 * c4 + 2) * HPC
                per = max(1, round(n_pairs / max(len(fillers), 1)))
                pi = 0

                def finish_head(h, pts):
                    # one-head-delayed O accumulation: by the time these run,
                    # their exps are a full head old and never stall the PE.
                    for qb in range(4):
                        emit_o_qb(c4, h, qb, pts)
                    if h == 1:
                        for tb in range(4):
                            emit_xbar(4 * c4 + tb, 0)
                    elif h == 3:
                        for tb in range(4):
                            emit_xbar(4 * c4 + tb, 1)

                prev = None
                for h in range(HPC):
                    pairs = [(2 * i, 2 * i + 1) for i in range(2 * c4 + 2)]
                    pts = []
                    for pair_idx, (ja, jb) in enumerate(pairs):
                        pts.append(emit_pair(c4, h, ja, jb))
                        gi = h * len(pairs) + pair_idx
                        if gi % per == per - 1 and pi < len(fillers):
                            f, args = fillers[pi]
                            f(*args)
                            pi += 1
                    if prev is not None:
                        finish_head(*prev)
                    prev = (h, pts)
                finish_head(*prev)
                while pi < len(fillers):
                    f, args = fillers[pi]
                    f(*args)
                    pi += 1

            # close attention pools first so their drains overlap the tail
            ph2b.__exit__(None, None, None)
            ph2a.__exit__(None, None, None)
            ph1pool.__exit__(None, None, None)

            # tail: out-proj of chunk 3 on a dedicated full-PSUM pool, split
            # fb0 (ready after h=1's xbars, schedules into ph2 idle slots)
            # from fb1 + copy + store.
            tailp = tc.tile_pool(name="ps_tail", bufs=8, space="PSUM")
            ps_tail = tailp.__enter__()
            tail_ps = {}
            for tb in range(12, 16):
                for oc in range(2):
                    ps = ps_tail.tile([128, 512], F32)
                    nc.tensor.matmul(
                        ps, attT_sb[:, 0, tb * 128:(tb + 1) * 128],
                        wo_sb[:, 0, oc * 512:(oc + 1) * 512],
                        start=True, stop=False,
                    )
                    tail_ps[(tb, oc)] = ps
            for tb in range(12, 16):
                zs = zs_pool.tile([128, 2, 512], BF16)
                for oc in range(2):
                    ps = tail_ps[(tb, oc)]
                    nc.tensor.matmul(
                        ps, attT_sb[:, 1, tb * 128:(tb + 1) * 128],
                        wo_sb[:, 1, oc * 512:(oc + 1) * 512],
                        start=False, stop=True,
                    )
                    if oc == 0:
                        nc.vector.tensor_copy(zs[:, oc, :], ps)
                    else:
                        nc.scalar.copy(zs[:, oc, :], ps)
                nc.sync.dma_start(
                    out=out[tb * 128:(tb + 1) * 128, :],
                    in_=zs.rearrange("p a b -> p (a b)"),
                )

            zsp.__exit__(None, None, None)
            tailp.__exit__(None, None, None)
    _split_excess_waits(nc)
    return nc


_NC = None


def _get_nc():
    global _NC
    if _NC is None:
        _NC = _build()
    return _NC


_BF16_NP = mybir.dt.np(BF16)


def make_in_maps(x, Wqkv, bqkv, Wo, bo):
    x = np.asarray(x, np.float32)
    Wqkv = np.asarray(Wqkv, np.float32)
    bqkv = np.asarray(bqkv, np.float32)
    Wo = np.asarray(Wo, np.float32)
    ident = np.eye(128, dtype=_BF16_NP)
    maskw = np.zeros((128, 512), np.float32)
    for p in range(128):
        maskw[p, :p] = MASK_NEG
    maskw = maskw.astype(_BF16_NP)
    in_maps = []
    for c in range(8):
        b, g = c // 4, c % 4
        cs = slice(g * 4 * HD, (g + 1) * 4 * HD)  # 256 head cols
        wq = Wqkv[:, 0:D][:, cs] * 0.125
        wk = Wqkv[:, D:2 * D][:, cs]
        wv = Wqkv[:, 2 * D:3 * D][:, cs]
        bq = bqkv[0:D][cs] * 0.125
        bk = bqkv[D:2 * D][cs]
        bvs = bqkv[2 * D:3 * D][cs]
        in_maps.append({
            "xT": np.ascontiguousarray(x[b].T).astype(_BF16_NP),
            "wqk": np.ascontiguousarray(
                np.concatenate([wq, wk], axis=1)).astype(_BF16_NP),
            "wv": np.ascontiguousarray(wv).astype(_BF16_NP),
            "bqk": np.ascontiguousarray(np.concatenate([bq, bk])),
            "bv": np.ascontiguousarray(bvs),
            "wo": np.ascontiguousarray(Wo[cs, :]).astype(_BF16_NP),
            "ident": ident,
            "maskw": maskw,
        })
    return in_maps


def run_spmd(in_maps, trace=False):
    from concourse.bass_utils import run_bass_kernel_spmd
    return run_bass_kernel_spmd(_get_nc(), in_maps, list(range(8)), trace=trace)


def kernel(x, mask, Wqkv, bqkv, Wo, bo):
    """Full inputs in, full output out. mask is always causal-tril; causality
    is implemented structurally on device."""
    bo = np.asarray(bo, np.float32)
    res = run_spmd(make_in_maps(x, Wqkv, bqkv, Wo, bo))
    outs = [np.asarray(res.results[c]["out"], np.float32) for c in range(8)]
    full = np.empty((B, S, D), np.float32)
    for b in range(B):
        full[b] = outs[4 * b + 0] + outs[4 * b + 1] + outs[4 * b + 2] + outs[4 * b + 3]
        full[b] += bo
    return full


# revision 6
# speedup vs baseline: 1.0029x; 1.0029x over previous
"""GPT causal attention block (B=2, S=2048, H=16, hd=64, d=1024),
sharded over 8 NeuronCores as (batch x head-group): core c -> batch c//4,
heads 4*(c%4) .. 4*(c%4)+3.  All matmuls in bf16 (fp32 PSUM accumulate).

Per-core device program:
  ph1: qkT = (Wqk*[1/8,1]).T @ x^T  [512, 2048] bf16 (q cols pre-scaled 1/8)
       v   = x @ Wv + bv            stored ones-augmented [128,16,4,65] bf16
  ph2 (O-layout): per (qchunk c4, head h):
       per k-block pair (2j, 2j+1), sharing one 2-bank PSUM tile:
         ST  = kT_h[j].T @ qT_h     [128 ktok, <=512 qtok] PSUM
         ST += I.T @ maskw          (-60 on masked diag region, via matmul)
         PT  = exp(ST)              bf16 SBUF (ACT engine, one exp per pair)
       per q-block qb (one strictly-sequential PSUM group per bank):
         O[qb] = sum_j PT_j[:, qb].T @ v_aug_j   [128 qtok, 65], col 64 = l
         att = O * (1/l)            DVE recip + per-partition tensor_scalar
  ph1 chunks 1..3 and ph3 chunks 0..2 are emitted interleaved into ph2 as PE
  filler (late ph2 windows are exp/ACT-bound; PE runs projections there).
  xbar: attT = att.T via XBAR DMA-transpose (no compute-engine cost)
  ph3: out = attT.T @ Wo -> bf16 out (bias bo and partial-sum on host)
Host sums the 4 row-parallel partials per batch and adds bo.
"""
import sys
import numpy as np

sys.path.insert(0, "/opt/trn_rl_repo")

import concourse.bass as bass
import concourse.mybir as mybir
import concourse.tile as tile

B, S, D, NH, HD = 2, 2048, 1024, 16, 64
HPC = 4            # heads per core
NKB = S // 128     # 16 k-blocks
NQC = S // 512     # 4 q-chunks
F32 = mybir.dt.float32
BF16 = mybir.dt.bfloat16
MASK_NEG = -60.0
MAX_WAITS = 1      # one sync-wait per NoOp; walrus limits are per-engine and tight


def _split_excess_waits(nc, max_waits=MAX_WAITS):
    """walrus CoreV3 rejects instructions with more than ~4 sync waits; move
    the excess onto same-engine NoOps inserted just before the instruction."""
    n_split = 0
    for blk in nc.m.functions[0].blocks:
        for idx in range(len(blk.instructions) - 1, -1, -1):
            inst = blk.instructions[idx]
            if isinstance(inst, mybir.InstISA) and inst.isa_opcode == 176:
                # EVENT_SEMAPHORE_RANGE_CLEAR mis-encodes for this walrus
                # ("ISA wrong length"); sems are re-zeroed by NRT per load.
                blk.instructions.pop(idx)
        idx = 0
        while idx < len(blk.instructions):
            inst = blk.instructions[idx]
            si = inst.sync_info
            lim = 0 if isinstance(inst, mybir.InstMatmult) else max_waits
            if si is not None and si.on_wait and len(si.on_wait) > lim:
                waits = list(si.on_wait)
                si.on_wait = waits[len(waits) - lim:] if lim else []
                rest = waits[:len(waits) - lim] if lim else waits
                for i in range(0, len(rest), max_waits):
                    nop = mybir.InstNoOp(
                        name=nc.get_next_instruction_name(),
                        sync_info=mybir.SyncInfo(
                            on_wait=rest[i:i + max_waits], on_update=[]
                        ),
                        bass_nofuse=True,
                        engine=inst.engine,
                    )
                    nc.register_instruction(nop)
                    blk.instructions.insert(idx, nop)
                    idx += 1
                n_split += 1
            idx += 1
    return n_split


def _build():
    nc = bass.Bass("TRN2", target_bir_lowering=False, debug=False, num_devices=8)
    xT = nc.declare_dram_parameter("xT", [D, S], BF16, isOutput=False)
    wqk = nc.declare_dram_parameter("wqk", [D, 512], BF16, isOutput=False)
    wv = nc.declare_dram_parameter("wv", [D, 256], BF16, isOutput=False)
    bqk = nc.declare_dram_parameter("bqk", [512], F32, isOutput=False)
    bv = nc.declare_dram_parameter("bv", [256], F32, isOutput=False)
    wo = nc.declare_dram_parameter("wo", [256, D], BF16, isOutput=False)
    ident = nc.declare_dram_parameter("ident", [128, 128], BF16, isOutput=False)
    maskw = nc.declare_dram_parameter("maskw", [128, 512], BF16, isOutput=False)
    out = nc.declare_dram_parameter("out", [S, D], BF16, isOutput=True)

    with tile.TileContext(nc) as tc:
        with (
            tc.tile_pool(name="singles", bufs=1) as singles,
            tc.tile_pool(name="xtp", bufs=2) as xtp,
            tc.tile_pool(name="pt", bufs=18) as ptp,
            tc.tile_pool(name="rl", bufs=4) as rlp,
        ):
            # ---- resident SBUF tensors ----
            wqk_sb = singles.tile([128, 8, 512], BF16)
            wv_sb = singles.tile([128, 8, 256], BF16)
            wo_sb = singles.tile([128, 2, D], BF16)
            qT_sb = singles.tile([128, 2, S], BF16)        # q, heads pair-stacked
            kT_sb = singles.tile([128, 2, S], BF16)
            v_sb = singles.tile([128, NKB, HPC, 65], BF16)  # ones-augmented v
            att_sb = singles.tile([128, NKB, 256], BF16)   # [tok, tb, h*64+e]
            attT_sb = singles.tile([128, 2, S], BF16)      # [feat, fb, tok]
            bqk_sb = singles.tile([128, 4], F32)           # per-feat-block bias col
            bv_sb = singles.tile([128, 256], F32)          # bv partition-bcast
            ident_sb = singles.tile([128, 128], BF16)
            maskw_sb = singles.tile([128, 512], BF16)

            # ---- phase-1 units (emitted directly for chunk 0, as ph2 filler
            # for chunks 1..3) ----
            ph1pool = tc.tile_pool(name="ps_qkv", bufs=2, space="PSUM")
            ps_qkv = ph1pool.__enter__()
            xts = {}

            def emit_x_dma(t):
                xt = xtp.tile([128, 8, 512], BF16, tag=f"xt{t % 2}")
                nc.sync.dma_start(
                    out=xt,
                    in_=xT[:, t * 512:(t + 1) * 512].rearrange(
                        "(db p) s -> p db s", p=128),
                )
                xts[t] = xt

            # critical-path first: qk weights for the first three feature
            # blocks and x chunk 0 as single merged DMAs (HWDGE per-DMA
            # overhead dominates the cold start), then the rest.
            nc.sync.dma_start(
                out=wqk_sb[:, :, 0:384],
                in_=wqk[:, 0:384].rearrange("(db p) c -> p db c", p=128))
            xt0 = xtp.tile([128, 8, 512], BF16, tag="xt0")
            nc.sync.dma_start(
                out=xt0[:, :, 0:256],
                in_=xT[:, 0:256].rearrange("(db p) s -> p db s", p=128))
            nc.sync.dma_start(
                out=xt0[:, :, 256:512],
                in_=xT[:, 256:512].rearrange("(db p) s -> p db s", p=128))
            xts[0] = xt0
            nc.sync.dma_start(out=bqk_sb, in_=bqk[:].rearrange("(blk p) -> p blk", p=128))
            nc.sync.dma_start(
                out=wqk_sb[:, :, 384:512],
                in_=wqk[:, 384:512].rearrange("(db p) c -> p db c", p=128))
            nc.sync.dma_start(
                out=wv_sb, in_=wv[:, :].rearrange("(db p) c -> p db c", p=128))
            nc.sync.dma_start(
                out=bv_sb,
                in_=bass.AP(tensor=bv[:].tensor, offset=bv[:].offset, ap=[[0, 128], [1, 256]]),
            )
            nc.sync.dma_start(out=ident_sb, in_=ident[:, :])
            nc.sync.dma_start(out=maskw_sb, in_=maskw[:, :])
            nc.sync.dma_start(
                out=wo_sb, in_=wo[:, :].rearrange("(fb p) c -> p fb c", p=128))
            nc.vector.memset(v_sb[:, :, :, 64:65], 1.0)

            def emit_qk_unit(t, fb, half=None):
                # fb 0,1: q pair-blocks; 2,3: k pair-blocks
                xt = xts[t]
                sl = slice(0, 512) if half is None else slice(half * 256, half * 256 + 256)
                ps = ps_qkv.tile([128, 512], F32, tag="u")
                for d in range(8):
                    nc.tensor.matmul(
                        ps[:, sl], wqk_sb[:, d, fb * 128:(fb + 1) * 128], xt[:, d, sl],
                        start=(d == 0), stop=(d == 7),
                    )
                dst = (qT_sb if fb < 2 else kT_sb)[
                    :, fb % 2, t * 512 + sl.start:t * 512 + sl.stop]
                nc.vector.tensor_scalar_add(dst, ps[:, sl], bqk_sb[:, fb:fb + 1])

            def emit_v_unit(t, tb):
                xt = xts[t]
                psu = ps_qkv.tile([128, 512], F32, tag="u")
                psv = psu[:, 0:256]
                for d in range(8):
                    nc.tensor.matmul(
                        psv, xt[:, d, tb * 128:(tb + 1) * 128], wv_sb[:, d, :],
                        start=(d == 0), stop=(d == 7),
                    )
                kb = t * 4 + tb
                nc.vector.tensor_add(
                    out=v_sb[:, kb, :, 0:64],
                    in0=psv.rearrange("p (h e) -> p h e", h=4),
                    in1=bv_sb.rearrange("p (h e) -> p h e", h=4),
                )

            # ---- phase-2 pools ----
            ph2a = tc.tile_pool(name="ps_st", bufs=2, space="PSUM")
            ph2b = tc.tile_pool(name="ps_o", bufs=2, space="PSUM")
            ps_st = ph2a.__enter__()
            ps_o = ph2b.__enter__()

            def emit_pair(c4, h, ja, jb):
                """ST+exp for k-blocks (ja, jb) against q-chunk c4 of head h.
                Returns the pt tile holding exp(scores)."""
                hp, hb = h // 2, (h % 2) * 64
                q_lo = c4 * 512
                los = []
                st = ps_st.tile([128, 2, 512], F32)
                for jj, j in ((0, ja), (1, jb)):
                    m = j - 4 * c4
                    lo = max(m, 0) * 128
                    los.append(lo)
                    nc.tensor.matmul(
                        st[:, jj, lo:512],
                        kT_sb[hb:hb + 64, hp, j * 128:(j + 1) * 128],
                        qT_sb[hb:hb + 64, hp, q_lo + lo:q_lo + 512],
                        start=True, stop=(m < 0),
                    )
                    if m >= 0:
                        # accumulate -60 onto the masked (strictly-lower) part
                        # of the diagonal 128x128 sub-block; exp then yields
                        # ~e^-60, i.e. an effective causal mask.
                        nc.tensor.matmul(
                            st[:, jj, lo:lo + 128], ident_sb, maskw_sb[:, 0:128],
                            start=False, stop=True,
                        )
                pt = ptp.tile([128, 2, 512], BF16)
                lo = min(los)
                nc.scalar.activation(
                    out=pt[:, :, lo:512], in_=st[:, :, lo:512],
                    func=mybir.ActivationFunctionType.Exp,
                )
                return pt

            def emit_o_qb(c4, h, qb, pts):
                """One PSUM accumulation group: O[qb] over all j <= 4*c4+qb.
                Strictly sequential group in its own bank (HW requirement)."""
                last_j = 4 * c4 + qb
                po = ps_o.tile([128, 65], F32)
                for j in range(last_j + 1):
                    pt = pts[j // 2]
                    nc.tensor.matmul(
                        po,
                        pt[:, j % 2, qb * 128:(qb + 1) * 128],
                        v_sb[:, j, h, :],
                        start=(j == 0), stop=(j == last_j),
                    )
                rl = rlp.tile([128, 1], F32)
                nc.vector.reciprocal(rl, po[:, 64:65])
                nc.vector.tensor_scalar_mul(
                    att_sb[:, 4 * c4 + qb, h * 64:(h + 1) * 64],
                    po[:, 0:64],
                    rl[:, 0:1],
                )

            def emit_xbar(tb, fb):
                nc.sync.dma_start_transpose(
                    out=attT_sb[:, fb, tb * 128:(tb + 1) * 128],
                    in_=att_sb[:, tb, fb * 128:(fb + 1) * 128],
                )

            zsp = tc.tile_pool(name="zs", bufs=4)
            zs_pool = zsp.__enter__()

            def emit_ph3_unit(tb, last):
                zs = zs_pool.tile([128, 2, 512], BF16)
                for oc in range(2):
                    ps = ps_qkv.tile([128, 512], F32, tag="u")
                    for fb in range(2):
                        nc.tensor.matmul(
                            ps, attT_sb[:, fb, tb * 128:(tb + 1) * 128],
                            wo_sb[:, fb, oc * 512:(oc + 1) * 512],
                            start=(fb == 0), stop=(fb == 1),
                        )
                    if last and oc == 1:
                        nc.scalar.copy(zs[:, oc, :], ps)
                    else:
                        nc.vector.tensor_copy(zs[:, oc, :], ps)
                nc.sync.dma_start(
                    out=out[tb * 128:(tb + 1) * 128, :],
                    in_=zs.rearrange("p a b -> p (a b)"),
                )

            # ---- schedule ----
            # chunk 0 qk at half-token granularity: the first attention pairs
            # only need the first half projected, so they start ~4us earlier.
            for fb, half in ((0, 0), (2, 0), (0, 1), (2, 1), (1, 0), (3, 0), (1, 1), (3, 1)):
                emit_qk_unit(0, fb, half)
            for tb in range(4):
                emit_v_unit(0, tb)
            emit_x_dma(1)

            # PE filler units dripped into each ACT-bound ph2 chunk:
            #   chunk 0,1: the qkv units of the next chunk
            #   chunk 2:   qkv units of chunk 3 + out-proj of chunks 0,1 (part)
            #   chunk 3:   out-proj through chunk 2
            # out-proj of chunk 3 runs at the end.
            for c4 in range(NQC):
                fillers = []
                if c4 + 2 < NQC:
                    emit_x_dma(c4 + 2)
                if c4 + 1 < NQC:
                    fillers += [(emit_qk_unit, (c4 + 1, fb)) for fb in range(4)]
                    fillers += [(emit_v_unit, (c4 + 1, tb)) for tb in range(4)]
                if c4 == 2:
                    fillers += [(emit_ph3_unit, (tb, False)) for tb in range(4)]
                elif c4 == 3:
                    fillers += [(emit_ph3_unit, (tb, False)) for tb in range(4, 12)]
                n_pairs = (2 * c4 + 2) * HPC
                per = max(1, round(n_pairs / max(len(fillers), 1)))
                pi = 0

                def finish_head(h, pts):
                    # one-head-delayed O accumulation: by the time these run,
                    # their exps are a full head old and never stall the PE.
                    for qb in range(4):
                        emit_o_qb(c4, h, qb, pts)
                    if h == 1:
                        for tb in range(4):
                            emit_xbar(4 * c4 + tb, 0)
                    elif h == 3:
                        for tb in range(4):
                            emit_xbar(4 * c4 + tb, 1)

                prev = None
                for h in range(HPC):
                    pairs = [(2 * i, 2 * i + 1) for i in range(2 * c4 + 2)]
                    pts = []
                    for pair_idx, (ja, jb) in enumerate(pairs):
                        pts.append(emit_pair(c4, h, ja, jb))
                        gi = h * len(pairs) + pair_idx
                        if gi % per == per - 1 and pi < len(fillers):
                            f, args = fillers[pi]
                            f(*args)
                            pi += 1
                    if prev is not None:
                        finish_head(*prev)
                    prev = (h, pts)
                finish_head(*prev)
                while pi < len(fillers):
                    f, args = fillers[pi]
                    f(*args)
                    pi += 1

            # close attention pools first so their drains overlap the tail
            ph2b.__exit__(None, None, None)
            ph2a.__exit__(None, None, None)
            ph1pool.__exit__(None, None, None)

            # tail: out-proj of chunk 3 on a dedicated full-PSUM pool, split
            # fb0 (ready after h=1's xbars, schedules into ph2 idle slots)
            # from fb1 + copy + store.
            tailp = tc.tile_pool(name="ps_tail", bufs=8, space="PSUM")
            ps_tail = tailp.__enter__()
            tail_ps = {}
            for tb in range(12, 16):
                for oc in range(2):
                    ps = ps_tail.tile([128, 512], F32)
                    nc.tensor.matmul(
                        ps, attT_sb[:, 0, tb * 128:(tb + 1) * 128],
                        wo_sb[:, 0, oc * 512:(oc + 1) * 512],
                        start=True, stop=False,
                    )
                    tail_ps[(tb, oc)] = ps
            for tb in range(12, 16):
                zs = zs_pool.tile([128, 2, 512], BF16)
                for oc in range(2):
                    ps = tail_ps[(tb, oc)]
                    nc.tensor.matmul(
                        ps, attT_sb[:, 1, tb * 128:(tb + 1) * 128],
                        wo_sb[:, 1, oc * 512:(oc + 1) * 512],
                        start=False, stop=True,
                    )
                    if oc == 0:
                        nc.vector.tensor_copy(zs[:, oc, :], ps)
                    else:
                        nc.scalar.copy(zs[:, oc, :], ps)
                nc.sync.dma_start(
                    out=out[tb * 128:(tb + 1) * 128, :],
                    in_=zs.rearrange("p a b -> p (a b)"),
                )

            zsp.__exit__(None, None, None)
            tailp.__exit__(None, None, None)
    _split_excess_waits(nc)
    return nc


_NC = None


def _get_nc():
    global _NC
    if _NC is None:
        _NC = _build()
    return _NC


_BF16_NP = mybir.dt.np(BF16)


def make_in_maps(x, Wqkv, bqkv, Wo, bo):
    x = np.asarray(x, np.float32)
    Wqkv = np.asarray(Wqkv, np.float32)
    bqkv = np.asarray(bqkv, np.float32)
    Wo = np.asarray(Wo, np.float32)
    ident = np.eye(128, dtype=_BF16_NP)
    maskw = np.zeros((128, 512), np.float32)
    for p in range(128):
        maskw[p, :p] = MASK_NEG
    maskw = maskw.astype(_BF16_NP)
    in_maps = []
    for c in range(8):
        b, g = c // 4, c % 4
        cs = slice(g * 4 * HD, (g + 1) * 4 * HD)  # 256 head cols
        wq = Wqkv[:, 0:D][:, cs] * 0.125
        wk = Wqkv[:, D:2 * D][:, cs]
        wv = Wqkv[:, 2 * D:3 * D][:, cs]
        bq = bqkv[0:D][cs] * 0.125
        bk = bqkv[D:2 * D][cs]
        bvs = bqkv[2 * D:3 * D][cs]
        in_maps.append({
            "xT": np.ascontiguousarray(x[b].T).astype(_BF16_NP),
            "wqk": np.ascontiguousarray(
                np.concatenate([wq, wk], axis=1)).astype(_BF16_NP),
            "wv": np.ascontiguousarray(wv).astype(_BF16_NP),
            "bqk": np.ascontiguousarray(np.concatenate([bq, bk])),
            "bv": np.ascontiguousarray(bvs),
            "wo": np.ascontiguousarray(Wo[cs, :]).astype(_BF16_NP),
            "ident": ident,
            "maskw": maskw,
        })
    return in_maps


def run_spmd(in_maps, trace=False):
    from concourse.bass_utils import run_bass_kernel_spmd
    return run_bass_kernel_spmd(_get_nc(), in_maps, list(range(8)), trace=trace)


def kernel(x, mask, Wqkv, bqkv, Wo, bo):
    """Full inputs in, full output out. mask is always causal-tril; causality
    is implemented structurally on device."""
    bo = np.asarray(bo, np.float32)
    res = run_spmd(make_in_maps(x, Wqkv, bqkv, Wo, bo))
    outs = [np.asarray(res.results[c]["out"], np.float32) for c in range(8)]
    full = np.empty((B, S, D), np.float32)
    for b in range(B):
        full[b] = outs[4 * b + 0] + outs[4 * b + 1] + outs[4 * b + 2] + outs[4 * b + 3]
        full[b] += bo
    return full


# revision 7
# speedup vs baseline: 1.1420x; 1.1386x over previous
"""GPT causal attention block (B=2, S=2048, H=16, hd=64, d=1024),
sharded over 8 NeuronCores as (batch x head-group): core c -> batch c//4,
heads 4*(c%4) .. 4*(c%4)+3.  All matmuls in bf16 (fp32 PSUM accumulate).

Per-core device program:
  ph1: qkT = (Wqk*[1/8,1]).T @ x^T  [512, 2048] bf16 (q cols pre-scaled 1/8)
       v   = x @ Wv + bv            stored ones-augmented [128,16,4,65] bf16
  ph2 (O-layout): per (qchunk c4, head h):
       per k-block pair (2j, 2j+1), sharing one 2-bank PSUM tile:
         ST  = kT_h[j].T @ qT_h     [128 ktok, <=512 qtok] PSUM
         ST += I.T @ maskw          (-60 on masked diag region, via matmul)
         PT  = exp(ST)              bf16 SBUF (ACT engine, one exp per pair)
       per q-block qb (one strictly-sequential PSUM group per bank):
         O[qb] = sum_j PT_j[:, qb].T @ v_aug_j   [128 qtok, 65], col 64 = l
         att = O * (1/l)            DVE recip + per-partition tensor_scalar
  ph1 chunks 1..3 and ph3 chunks 0..2 are emitted interleaved into ph2 as PE
  filler (late ph2 windows are exp/ACT-bound; PE runs projections there).
  xbar: attT = att.T via XBAR DMA-transpose (no compute-engine cost)
  ph3: out = attT.T @ Wo -> bf16 out (bias bo and partial-sum on host)
Host sums the 4 row-parallel partials per batch and adds bo.
"""
import sys
import numpy as np

sys.path.insert(0, "/opt/trn_rl_repo")

import concourse.bass as bass
import concourse.mybir as mybir
import concourse.tile as tile

B, S, D, NH, HD = 2, 2048, 1024, 16, 64
HPC = 4            # heads per core
NKB = S // 128     # 16 k-blocks
NQC = S // 512     # 4 q-chunks
F32 = mybir.dt.float32
BF16 = mybir.dt.bfloat16
MASK_NEG = -60.0
MAX_WAITS = 1      # one sync-wait per NoOp; walrus limits are per-engine and tight


def _split_excess_waits(nc, max_waits=MAX_WAITS):
    """walrus CoreV3 rejects instructions with more than ~4 sync waits; move
    the excess onto same-engine NoOps inserted just before the instruction."""
    n_split = 0
    for blk in nc.m.functions[0].blocks:
        for idx in range(len(blk.instructions) - 1, -1, -1):
            inst = blk.instructions[idx]
            if isinstance(inst, mybir.InstISA) and inst.isa_opcode == 176:
                # EVENT_SEMAPHORE_RANGE_CLEAR mis-encodes for this walrus
                # ("ISA wrong length"); sems are re-zeroed by NRT per load.
                blk.instructions.pop(idx)
        idx = 0
        while idx < len(blk.instructions):
            inst = blk.instructions[idx]
            si = inst.sync_info
            lim = 0 if isinstance(inst, mybir.InstMatmult) else max_waits
            if si is not None and si.on_wait and len(si.on_wait) > lim:
                waits = list(si.on_wait)
                si.on_wait = waits[len(waits) - lim:] if lim else []
                rest = waits[:len(waits) - lim] if lim else waits
                for i in range(0, len(rest), max_waits):
                    nop = mybir.InstNoOp(
                        name=nc.get_next_instruction_name(),
                        sync_info=mybir.SyncInfo(
                            on_wait=rest[i:i + max_waits], on_update=[]
                        ),
                        bass_nofuse=True,
                        engine=inst.engine,
                    )
                    nc.register_instruction(nop)
                    blk.instructions.insert(idx, nop)
                    idx += 1
                n_split += 1
            idx += 1
    return n_split


def _build():
    nc = bass.Bass("TRN2", target_bir_lowering=False, debug=False, num_devices=8)
    xT = nc.declare_dram_parameter("xT", [D, S], BF16, isOutput=False)
    wqk = nc.declare_dram_parameter("wqk", [D, 512], BF16, isOutput=False)
    wv = nc.declare_dram_parameter("wv", [D, 256], BF16, isOutput=False)
    bqk = nc.declare_dram_parameter("bqk", [512], F32, isOutput=False)
    bv = nc.declare_dram_parameter("bv", [256], F32, isOutput=False)
    wo = nc.declare_dram_parameter("wo", [256, D], BF16, isOutput=False)
    ident = nc.declare_dram_parameter("ident", [128, 128], BF16, isOutput=False)
    maskw = nc.declare_dram_parameter("maskw", [128, 512], BF16, isOutput=False)
    out = nc.declare_dram_parameter("out", [S, D], BF16, isOutput=True)

    with tile.TileContext(nc) as tc:
        with (
            tc.tile_pool(name="singles", bufs=1) as singles,
            tc.tile_pool(name="xtp", bufs=2) as xtp,
            tc.tile_pool(name="pt", bufs=18) as ptp,
            tc.tile_pool(name="rl", bufs=4) as rlp,
        ):
            # ---- resident SBUF tensors ----
            wqk_sb = singles.tile([128, 8, 512], BF16)
            wv_sb = singles.tile([128, 8, 256], BF16)
            wo_sb = singles.tile([128, 2, D], BF16)
            qT_sb = singles.tile([128, 2, S], BF16)        # q, heads pair-stacked
            kT_sb = singles.tile([128, 2, S], BF16)
            v_sb = singles.tile([128, NKB, HPC, 65], BF16)  # ones-augmented v
            att_sb = singles.tile([128, NKB, 256], BF16)   # [tok, tb, h*64+e]
            attT_sb = singles.tile([128, 2, S], BF16)      # [feat, fb, tok]
            bqk_sb = singles.tile([128, 4], F32)           # per-feat-block bias col
            bv_sb = singles.tile([128, 256], F32)          # bv partition-bcast
            ident_sb = singles.tile([128, 128], BF16)
            maskw_sb = singles.tile([128, 512], BF16)

            # ---- phase-1 units (emitted directly for chunk 0, as ph2 filler
            # for chunks 1..3) ----
            ph1pool = tc.tile_pool(name="ps_qkv", bufs=2, space="PSUM")
            ps_qkv = ph1pool.__enter__()
            xts = {}

            def emit_x_dma(t):
                xt = xtp.tile([128, 8, 512], BF16, tag=f"xt{t % 2}")
                nc.sync.dma_start(
                    out=xt,
                    in_=xT[:, t * 512:(t + 1) * 512].rearrange(
                        "(db p) s -> p db s", p=128),
                )
                xts[t] = xt

            # critical-path first: qk weights for the first three feature
            # blocks and x chunk 0 as single merged DMAs (HWDGE per-DMA
            # overhead dominates the cold start), then the rest.
            nc.sync.dma_start(
                out=wqk_sb[:, :, 0:128],
                in_=wqk[:, 0:128].rearrange("(db p) c -> p db c", p=128))
            xt0 = xtp.tile([128, 8, 512], BF16, tag="xt0")
            nc.sync.dma_start(
                out=xt0[:, :, 0:256],
                in_=xT[:, 0:256].rearrange("(db p) s -> p db s", p=128))
            nc.sync.dma_start(
                out=xt0[:, :, 256:512],
                in_=xT[:, 256:512].rearrange("(db p) s -> p db s", p=128))
            xts[0] = xt0
            nc.sync.dma_start(
                out=wqk_sb[:, :, 128:384],
                in_=wqk[:, 128:384].rearrange("(db p) c -> p db c", p=128))
            nc.sync.dma_start(out=bqk_sb, in_=bqk[:].rearrange("(blk p) -> p blk", p=128))
            nc.sync.dma_start(
                out=wqk_sb[:, :, 384:512],
                in_=wqk[:, 384:512].rearrange("(db p) c -> p db c", p=128))
            nc.sync.dma_start(
                out=wv_sb, in_=wv[:, :].rearrange("(db p) c -> p db c", p=128))
            nc.sync.dma_start(
                out=bv_sb,
                in_=bass.AP(tensor=bv[:].tensor, offset=bv[:].offset, ap=[[0, 128], [1, 256]]),
            )
            nc.sync.dma_start(out=ident_sb, in_=ident[:, :])
            nc.sync.dma_start(out=maskw_sb, in_=maskw[:, :])
            nc.sync.dma_start(
                out=wo_sb, in_=wo[:, :].rearrange("(fb p) c -> p fb c", p=128))
            nc.vector.memset(v_sb[:, :, :, 64:65], 1.0)

            def emit_qk_unit(t, fb, half=None):
                # fb 0,1: q pair-blocks; 2,3: k pair-blocks
                xt = xts[t]
                sl = slice(0, 512) if half is None else slice(half * 256, half * 256 + 256)
                ps = ps_qkv.tile([128, 512], F32, tag="u")
                for d in range(8):
                    nc.tensor.matmul(
                        ps[:, sl], wqk_sb[:, d, fb * 128:(fb + 1) * 128], xt[:, d, sl],
                        start=(d == 0), stop=(d == 7),
                    )
                dst = (qT_sb if fb < 2 else kT_sb)[
                    :, fb % 2, t * 512 + sl.start:t * 512 + sl.stop]
                nc.vector.tensor_scalar_add(dst, ps[:, sl], bqk_sb[:, fb:fb + 1])

            def emit_v_unit(t, tb):
                xt = xts[t]
                psu = ps_qkv.tile([128, 512], F32, tag="u")
                psv = psu[:, 0:256]
                for d in range(8):
                    nc.tensor.matmul(
                        psv, xt[:, d, tb * 128:(tb + 1) * 128], wv_sb[:, d, :],
                        start=(d == 0), stop=(d == 7),
                    )
                kb = t * 4 + tb
                nc.vector.tensor_add(
                    out=v_sb[:, kb, :, 0:64],
                    in0=psv.rearrange("p (h e) -> p h e", h=4),
                    in1=bv_sb.rearrange("p (h e) -> p h e", h=4),
                )

            # ---- phase-2 pools ----
            ph2a = tc.tile_pool(name="ps_st", bufs=2, space="PSUM")
            ph2b = tc.tile_pool(name="ps_o", bufs=2, space="PSUM")
            ps_st = ph2a.__enter__()
            ps_o = ph2b.__enter__()

            def emit_pair(c4, h, ja, jb):
                """ST+exp for k-blocks (ja, jb) against q-chunk c4 of head h.
                Returns the pt tile holding exp(scores)."""
                hp, hb = h // 2, (h % 2) * 64
                q_lo = c4 * 512
                los = []
                st = ps_st.tile([128, 2, 512], F32)
                for jj, j in ((0, ja), (1, jb)):
                    m = j - 4 * c4
                    lo = max(m, 0) * 128
                    los.append(lo)
                    nc.tensor.matmul(
                        st[:, jj, lo:512],
                        kT_sb[hb:hb + 64, hp, j * 128:(j + 1) * 128],
                        qT_sb[hb:hb + 64, hp, q_lo + lo:q_lo + 512],
                        start=True, stop=(m < 0),
                    )
                    if m >= 0:
                        # accumulate -60 onto the masked (strictly-lower) part
                        # of the diagonal 128x128 sub-block; exp then yields
                        # ~e^-60, i.e. an effective causal mask.
                        nc.tensor.matmul(
                            st[:, jj, lo:lo + 128], ident_sb, maskw_sb[:, 0:128],
                            start=False, stop=True,
                        )
                pt = ptp.tile([128, 2, 512], BF16)
                lo = min(los)
                nc.scalar.activation(
                    out=pt[:, :, lo:512], in_=st[:, :, lo:512],
                    func=mybir.ActivationFunctionType.Exp,
                )
                return pt

            def emit_o_qb(c4, h, qb, pts):
                """One PSUM accumulation group: O[qb] over all j <= 4*c4+qb.
                Strictly sequential group in its own bank (HW requirement)."""
                last_j = 4 * c4 + qb
                po = ps_o.tile([128, 65], F32)
                for j in range(last_j + 1):
                    pt = pts[j // 2]
                    nc.tensor.matmul(
                        po,
                        pt[:, j % 2, qb * 128:(qb + 1) * 128],
                        v_sb[:, j, h, :],
                        start=(j == 0), stop=(j == last_j),
                    )
                rl = rlp.tile([128, 1], F32)
                nc.vector.reciprocal(rl, po[:, 64:65])
                nc.vector.tensor_scalar_mul(
                    att_sb[:, 4 * c4 + qb, h * 64:(h + 1) * 64],
                    po[:, 0:64],
                    rl[:, 0:1],
                )

            def emit_xbar(tb, fb):
                nc.sync.dma_start_transpose(
                    out=attT_sb[:, fb, tb * 128:(tb + 1) * 128],
                    in_=att_sb[:, tb, fb * 128:(fb + 1) * 128],
                )

            zsp = tc.tile_pool(name="zs", bufs=4)
            zs_pool = zsp.__enter__()

            def emit_ph3_unit(tb, last):
                zs = zs_pool.tile([128, 2, 512], BF16)
                for oc in range(2):
                    ps = ps_qkv.tile([128, 512], F32, tag="u")
                    for fb in range(2):
                        nc.tensor.matmul(
                            ps, attT_sb[:, fb, tb * 128:(tb + 1) * 128],
                            wo_sb[:, fb, oc * 512:(oc + 1) * 512],
                            start=(fb == 0), stop=(fb == 1),
                        )
                    if last and oc == 1:
                        nc.scalar.copy(zs[:, oc, :], ps)
                    else:
                        nc.vector.tensor_copy(zs[:, oc, :], ps)
                nc.sync.dma_start(
                    out=out[tb * 128:(tb + 1) * 128, :],
                    in_=zs.rearrange("p a b -> p (a b)"),
                )

            # ---- schedule ----
            # chunk 0 qk at half-token granularity: the first attention pairs
            # only need the first half projected, so they start ~4us earlier.
            for fb, half in ((0, 0), (2, 0), (0, 1), (2, 1), (1, 0), (3, 0), (1, 1), (3, 1)):
                emit_qk_unit(0, fb, half)
            for tb in range(4):
                emit_v_unit(0, tb)
            emit_x_dma(1)

            # PE filler units dripped into each ACT-bound ph2 chunk:
            #   chunk 0,1: the qkv units of the next chunk
            #   chunk 2:   qkv units of chunk 3 + out-proj of chunks 0,1 (part)
            #   chunk 3:   out-proj through chunk 2
            # out-proj of chunk 3 runs at the end.
            for c4 in range(NQC):
                fillers = []
                if c4 + 2 < NQC:
                    emit_x_dma(c4 + 2)
                if c4 + 1 < NQC:
                    fillers += [(emit_qk_unit, (c4 + 1, fb)) for fb in range(4)]
                    fillers += [(emit_v_unit, (c4 + 1, tb)) for tb in range(4)]
                if c4 == 2:
                    fillers += [(emit_ph3_unit, (tb, False)) for tb in range(4)]
                elif c4 == 3:
                    fillers += [(emit_ph3_unit, (tb, False)) for tb in range(4, 12)]
                n_pairs = (2 * c4 + 2) * HPC
                per = max(1, round(n_pairs / max(len(fillers), 1)))
                pi = 0

                def finish_head(h, pts):
                    # one-head-delayed O accumulation: by the time these run,
                    # their exps are a full head old and never stall the PE.
                    for qb in range(4):
                        emit_o_qb(c4, h, qb, pts)
                    if h == 1:
                        for tb in range(4):
                            emit_xbar(4 * c4 + tb, 0)
                    elif h == 3:
                        for tb in range(4):
                            emit_xbar(4 * c4 + tb, 1)

                prev = None
                for h in range(HPC):
                    pairs = [(2 * i, 2 * i + 1) for i in range(2 * c4 + 2)]
                    pts = []
                    for pair_idx, (ja, jb) in enumerate(pairs):
                        pts.append(emit_pair(c4, h, ja, jb))
                        gi = h * len(pairs) + pair_idx
                        if gi % per == per - 1 and pi < len(fillers):
                            f, args = fillers[pi]
                            f(*args)
                            pi += 1
                    if prev is not None:
                        finish_head(*prev)
                    prev = (h, pts)
                finish_head(*prev)
                while pi < len(fillers):
                    f, args = fillers[pi]
                    f(*args)
                    pi += 1

            # close attention pools first so their drains overlap the tail
            ph2b.__exit__(None, None, None)
            ph2a.__exit__(None, None, None)
            ph1pool.__exit__(None, None, None)

            # tail: out-proj of chunk 3 on a dedicated full-PSUM pool, split
            # fb0 (ready after h=1's xbars, schedules into ph2 idle slots)
            # from fb1 + copy + store.
            tailp = tc.tile_pool(name="ps_tail", bufs=8, space="PSUM")
            ps_tail = tailp.__enter__()
            tail_ps = {}
            for tb in range(12, 16):
                for oc in range(2):
                    ps = ps_tail.tile([128, 512], F32)
                    nc.tensor.matmul(
                        ps, attT_sb[:, 0, tb * 128:(tb + 1) * 128],
                        wo_sb[:, 0, oc * 512:(oc + 1) * 512],
                        start=True, stop=False,
                    )
                    tail_ps[(tb, oc)] = ps
            for tb in range(12, 16):
                zs = zs_pool.tile([128, 2, 512], BF16)
                for oc in range(2):
                    ps = tail_ps[(tb, oc)]
                    nc.tensor.matmul(
                        ps, attT_sb[:, 1, tb * 128:(tb + 1) * 128],
                        wo_sb[:, 1, oc * 512:(oc + 1) * 512],
                        start=False, stop=True,
                    )
                    if oc == 0:
                        nc.vector.tensor_copy(zs[:, oc, :], ps)
                    else:
                        nc.scalar.copy(zs[:, oc, :], ps)
                nc.sync.dma_start(
                    out=out[tb * 128:(tb + 1) * 128, :],
                    in_=zs.rearrange("p a b -> p (a b)"),
                )

            zsp.__exit__(None, None, None)
            tailp.__exit__(None, None, None)
    _split_excess_waits(nc)
    return nc


_NC = None


def _get_nc():
    global _NC
    if _NC is None:
        _NC = _build()
    return _NC


_BF16_NP = mybir.dt.np(BF16)


def make_in_maps(x, Wqkv, bqkv, Wo, bo):
    x = np.asarray(x, np.float32)
    Wqkv = np.asarray(Wqkv, np.float32)
    bqkv = np.asarray(bqkv, np.float32)
    Wo = np.asarray(Wo, np.float32)
    ident = np.eye(128, dtype=_BF16_NP)
    maskw = np.zeros((128, 512), np.float32)
    for p in range(128):
        maskw[p, :p] = MASK_NEG
    maskw = maskw.astype(_BF16_NP)
    in_maps = []
    for c in range(8):
        b, g = c // 4, c % 4
        cs = slice(g * 4 * HD, (g + 1) * 4 * HD)  # 256 head cols
        wq = Wqkv[:, 0:D][:, cs] * 0.125
        wk = Wqkv[:, D:2 * D][:, cs]
        wv = Wqkv[:, 2 * D:3 * D][:, cs]
        bq = bqkv[0:D][cs] * 0.125
        bk = bqkv[D:2 * D][cs]
        bvs = bqkv[2 * D:3 * D][cs]
        in_maps.append({
            "xT": np.ascontiguousarray(x[b].T).astype(_BF16_NP),
            "wqk": np.ascontiguousarray(
                np.concatenate([wq, wk], axis=1)).astype(_BF16_NP),
            "wv": np.ascontiguousarray(wv).astype(_BF16_NP),
            "bqk": np.ascontiguousarray(np.concatenate([bq, bk])),
            "bv": np.ascontiguousarray(bvs),
            "wo": np.ascontiguousarray(Wo[cs, :]).astype(_BF16_NP),
            "ident": ident,
            "maskw": maskw,
        })
    return in_maps


def run_spmd(in_maps, trace=False):
    from concourse.bass_utils import run_bass_kernel_spmd
    return run_bass_kernel_spmd(_get_nc(), in_maps, list(range(8)), trace=trace)


def kernel(x, mask, Wqkv, bqkv, Wo, bo):
    """Full inputs in, full output out. mask is always causal-tril; causality
    is implemented structurally on device."""
    bo = np.asarray(bo, np.float32)
    res = run_spmd(make_in_maps(x, Wqkv, bqkv, Wo, bo))
    outs = [np.asarray(res.results[c]["out"], np.float32) for c in range(8)]
    full = np.empty((B, S, D), np.float32)
    for b in range(B):
        full[b] = outs[4 * b + 0] + outs[4 * b + 1] + outs[4 * b + 2] + outs[4 * b + 3]
        full[b] += bo
    return full
